# revision 27
# baseline (speedup 1.0000x reference)
"""Single-head causal attention on 8 trn2 NeuronCores (one batch element per core).

Problem: x [8, 2048, 1024], Wq/Wk/Wv [1024, 64] -> out [8, 2048, 64]
  q = x@Wq; k = x@Wk; v = x@Wv; out = causal_softmax(q k^T / sqrt(64)) @ v

Strategy (per core, batch-parallel across the 8 cores):
  - Host pre-packs each core's x^T into the exact SBUF image, s-block-major,
    so every DMA descriptor is a 4-8KB contiguous read. Weights load first
    (they gate the first projection), then the four 1MB x slabs stream with
    partitions 0:64 on the sync HWDGE queue and 64:128 on the scalar one.
  - Projections on the PE as fp16 matmuls: Q^T and K^T are produced packed
    ([Wq|Wk] weights). The partition-swapped copy (K^T on rows 0:64 / Q^T on
    rows 64:128, needed to pack score matmuls into both PE row groups) is
    made by partition-shifted engine copies straight out of the projection
    PSUM (DVE) and qk (GpSimd) — no DMA.
  - Scores are computed transposed (P^T[kv, q]) so softmax normalization can
    ride the PV matmul: V is augmented with a ones column, making rows 64:128
    of the PV output the softmax denominator. No max-subtraction is needed.
  - exp on ScalarE straight out of PSUM with the 1/sqrt(D) scale folded in.
  - Causal masking: score/PV matmuls are narrowed to 128-column granularity
    and a multiplicative 0/1 step mask cleans the diagonal tiles after exp.
  - Each block's pipeline is split into a scores-stream (PE scores -> ACT exp
    -> DVE mask -> pt tiles in SBUF) and a deferred PV chain, with the next
    block's projection interleaved into the scores-stream. The PE therefore
    never blocks in-order on an exp, and ACT (the 27us-busy engine) stays fed.
  - PV accumulates out^T in PSUM; normalizer divide is ACT Ln/Exp reciprocal
    + one DVE multiply. y is stored fp16 (host upcasts): halves the output DMA.
"""

import numpy as np

import concourse.bass as bass
import concourse.mybir as mybir
import concourse.tile as tile
from concourse.vector_clock import ScopedClock

S = 2048  # sequence length
E = 1024  # embed dim
D = 64    # head size
B = 8     # batch == number of cores
P = 128   # SBUF partitions
SBLK = 512         # q-block / s-block width (max fp32 matmul moving dim)
EC = E // P        # 8 contraction chunks
NSB = S // SBLK    # 4 s-blocks
NJT = S // P       # 16 kv tiles

f32 = mybir.dt.float32
f16 = mybir.dt.float16
MMDT = f16          # dtype of all large-matmul operands
MMNP = np.float16   # matching numpy dtype for host-side prep
AF = mybir.ActivationFunctionType

NWARM = 19            # PE warm-up matmuls (DVFS ramp) while slab 0 streams

_PATCHED = False


def _patch_tile_drain():
    """The walrus build in this container rejects instructions carrying more
    than one sem wait on the Tile exit Drain. Split the waits across a chain
    of drains, one wait each."""
    global _PATCHED
    if _PATCHED:
        return
    _PATCHED = True

    def _drain_and_barrier(self, tick_clock, wait_clock):
        drain_inst = self.nc.sync.drain()
        wait_clock.add_sem_waits(
            drain_inst.ins, ScopedClock({None: tick_clock.global_clock})
        )
        ins = drain_inst.ins
        si = ins.sync_info
        if si is not None and si.on_wait is not None and len(si.on_wait) > 1:
            waits = list(si.on_wait)
            ins.sync_info = mybir.SyncInfo(
                on_wait=[waits[0]], on_update=list(si.on_update or [])
            )
            for w in waits[1:]:
                d2 = self.nc.sync.drain()
                d2.ins.sync_info = mybir.SyncInfo(on_wait=[w], on_update=[])
        self.nc.all_engine_barrier()
        assert self.sems is not None
        popped = self.nc._tile_sem_poison_stack.pop()
        assert popped is self._sem_poison
        self.nc.clear_and_free_semaphores(list(self.sems.allocated().values()))
        self.nc.all_engine_barrier()

    tile.TileContext._drain_and_barrier = _drain_and_barrier


def _split_multiwaits(nc):
    """This container's walrus rejects instructions carrying more than one
    sem wait (setupSyncWait: 'Too many sync wait commands'). Hoist all but
    the last wait of every instruction onto same-engine NoOps placed
    immediately before it — the engine sequencer processes them in order,
    which is semantically identical."""
    ctr = 0
    for f in nc.m.functions:
        for bb in f.blocks:
            out = []
            changed = False
            for inst in bb.instructions:
                si = inst.sync_info
                if si is not None and si.on_wait is not None and len(si.on_wait) > 1:
                    waits = list(si.on_wait)
                    for w in waits[:-1]:
                        nop = mybir.InstNoOp(name=f"I-waitsplit-{ctr}")
                        ctr += 1
                        nop.engine = inst.engine
                        nop.sync_info = mybir.SyncInfo(on_wait=[w], on_update=[])
                        out.append(nop)
                    inst.sync_info = mybir.SyncInfo(
                        on_wait=[waits[-1]], on_update=list(si.on_update or [])
                    )
                    changed = True
                out.append(inst)
            if changed:
                bb.instructions = out


def _attention(ctx, tc, xt, y):
    nc = tc.nc
    scale = 1.0 / np.sqrt(D)

    persist = ctx.enter_context(tc.tile_pool(name="persist", bufs=1))
    xpool = ctx.enter_context(tc.tile_pool(name="xts", bufs=1))
    ppool = ctx.enter_context(tc.tile_pool(name="pp", bufs=15))
    rpool = ctx.enter_context(tc.tile_pool(name="rec", bufs=4))
    psproj = ctx.enter_context(tc.tile_pool(name="psproj", bufs=2, space="PSUM"))
    psscore = ctx.enter_context(tc.tile_pool(name="psscore", bufs=2, space="PSUM"))
    pspv = ctx.enter_context(tc.tile_pool(name="pspv", bufs=2, space="PSUM"))

    # ---- input stream: one 3KB/partition weights DMA on sync (it gates
    # proj0), x partition-halves split across both HWDGE queues; the scalar
    # queue starts slab 0 immediately.
    # The whole input (x slabs + weights) lives in ONE tile so each
    # dma_start covers one long contiguous run per partition — the per-queue
    # HWDGE rate is dispatch-limited (~60ns/descriptor), so descriptor size
    # directly sets stream bandwidth. Host layout per partition:
    # [slab0 8KB | wqkv 3KB | slab1 8KB | slab2 8KB | slab3 8KB].
    NW = EC * 3 * D                      # weight elements per partition
    NX = EC * SBLK                       # slab elements per partition
    xall = xpool.tile([P, 4 * NX + NW], MMDT, tag="xall")
    xoff = [0, NX + NW, 2 * NX + NW, 3 * NX + NW]
    xts = [
        xall[:, xoff[b] : xoff[b] + NX].rearrange("p (c s) -> p c s", c=EC)
        for b in range(NSB)
    ]
    wqkv_sb = xall[:, NX : NX + NW].rearrange("p (c m) -> p c m", c=EC)
    HP = P // 2

    # DMA split, tuned to BOTH the hardware and the Tile scheduler's model
    # (which charges each dma_start's whole transfer to the issuing engine
    # and serializes all transfers on one global resource):
    #  - scalar may only carry DMAs that are FIRST in the global order, so
    #    its modeled busy-window ends before any ACT work: s0hi + s1hi.
    #  - sync carries s0lo+weights, s1lo, s23lo.
    #  - gpsimd SWDGE (fast at 165 B/ns but ~6us gen latency, and a ~9.5us
    #    engine preamble) carries the late-needed s23hi; its dma_starts come
    #    before its constant setup so descriptor gen starts ASAP.
    S01 = NX + NW            # end of slab0+weights region
    S12 = 2 * NX + NW        # end of slab1 region
    nc.sync.dma_start(xall[:, :S01], xt[:, :S01])
    nc.sync.dma_start(xall[:HP, S01:S12], xt[:HP, S01:S12])
    nc.sync.dma_start(xall[:HP, S12:], xt[:HP, S12:])
    nc.gpsimd.dma_start(xall[HP:, S01:S12], xt[HP:, S01:S12])
    nc.gpsimd.dma_start(xall[HP:, S12:], xt[HP:, S12:])

    ident = persist.tile([P, P], f32, tag="ident")
    nc.gpsimd.memset(ident[:], 0.0)
    nc.gpsimd.affine_select(
        out=ident[:], in_=ident[:],
        compare_op=mybir.AluOpType.not_equal, fill=1.0,
        base=0, pattern=[[-1, P]], channel_multiplier=1,
    )
    maskW = persist.tile([P, 2 * SBLK], f32, tag="maskw")
    nc.gpsimd.memset(maskW[:], 1.0)
    nc.gpsimd.affine_select(
        out=maskW[:], in_=maskW[:],
        compare_op=mybir.AluOpType.is_ge, fill=0.0,
        base=-SBLK, pattern=[[1, 2 * SBLK]], channel_multiplier=-1,
    )


    # ---- PE warm-up: ramp the DVFS clock while slab 0 streams in --------
    warm_in = persist.tile([P, SBLK], MMDT, tag="warm")
    nc.vector.memset(warm_in[:], 0.25)
    wt = psproj.tile([P, SBLK], f32, tag="proj")
    for _ in range(NWARM):
        nc.tensor.matmul(wt[:], warm_in[:, :P], warm_in[:], start=True, stop=True)

    # ---- constants (gpsimd parts emitted above, before its DMAs) ---------
    ident16 = persist.tile([P, P], MMDT, tag="ident16")
    nc.vector.tensor_copy(ident16[:], ident[:])
    mask16 = persist.tile([P, 2 * SBLK], MMDT, tag="mask16")
    nc.vector.tensor_copy(mask16[:], maskW[:])

    # ---- persistent activations -----------------------------------------
    # qk:  rows 0:64 = Q^T, rows 64:128 = K^T (straight from packed psum)
    # qk2: rows 0:64 = K^T, rows 64:128 = Q^T (partition-shifted engine copies)
    qk = persist.tile([P, S], MMDT, tag="qk")
    qk2 = persist.tile([P, S], MMDT, tag="qk2")
    vT = persist.tile([D, S], MMDT, tag="vt")
    vAug = persist.tile([P, NJT, 2 * D], MMDT, tag="vaug")
    yT = persist.tile([D, S], MMDT, tag="ytout")
    ones_f32 = persist.tile([P, NJT, D], f32, tag="ones")
    nc.vector.memset(ones_f32[:], 1.0)
    nc.vector.tensor_copy(vAug[:, :, D:], ones_f32[:])

    def proj_chunks(b):
        """Emit-steps for s-block b's projections; the scores-stream of
        block b-1 interleaves these between its pairs so the PE always has
        independent matmuls queued behind exp-dependent ones."""
        sl = slice(b * SBLK, (b + 1) * SBLK)
        steps = []
        psQK = psproj.tile([P, SBLK], f32, tag="proj")
        psV = psproj.tile([P, SBLK], f32, tag="proj")

        def qk_mms(e0, psQK=psQK):
            for e in (e0, e0 + 1):
                nc.tensor.matmul(
                    psQK[:], wqkv_sb[:, e, : 2 * D], xts[b][:, e, :],
                    start=(e == 0), stop=(e == EC - 1),
                )

        def v_mms(e0, psV=psV):
            for e in (e0, e0 + 1):
                nc.tensor.matmul(
                    psV[:D, :], wqkv_sb[:, e, 2 * D :], xts[b][:, e, :],
                    start=(e == 0), stop=(e == EC - 1),
                )

        def qk_out():
            # all three on DVE (partition-shifted reads straight from PSUM):
            # gpsimd is ~2.7x slower per copy and now carries DMA, and the
            # scheduler models ACT/gpsimd DMA-issuers as long-busy
            nc.vector.tensor_copy(qk2[:D, sl], psQK[D:P, :])
            nc.vector.tensor_copy(qk[:, sl], psQK[:])
            nc.vector.tensor_copy(qk2[D:P, sl], psQK[:D, :])

        def v_out():
            nc.vector.tensor_copy(vT[:, sl], psV[:D, :])

        if b == 0:
            # all QK chunks first: the scores/exp pipeline start is gated on
            # them, while V is only needed by the (much later) PV chain
            for e0 in (0, 2, 4, 6):
                steps.append(lambda e0=e0: qk_mms(e0))
            steps.append(qk_out)
            for e0 in (0, 2, 4, 6):
                steps.append(lambda e0=e0: v_mms(e0))
            steps.append(v_out)
        else:
            steps.append(lambda: qk_mms(0))
            steps.append(lambda: qk_mms(2))
            steps.append(lambda: v_mms(0))
            steps.append(lambda: v_mms(2))
            steps.append(lambda: qk_mms(4))
            steps.append(lambda: qk_mms(6))
            steps.append(lambda: v_mms(4))
            steps.append(lambda: v_mms(6))
            steps.append(qk_out)
            steps.append(v_out)

        def v_tr(t):
            j = 4 * b + t
            psv_t = psproj.tile([P, SBLK], MMDT, tag="proj")
            nc.tensor.transpose(
                psv_t[:, :D], vT[:, j * P : (j + 1) * P], ident16[:D, :D]
            )
            nc.vector.tensor_copy(vAug[:, j, :D], psv_t[:, :D])

        for t in range(4):
            steps.append(lambda t=t: v_tr(t))
        return steps

    def proj(b):
        for step in proj_chunks(b):
            step()

    def scores_stream(b, bg=()):
        """PE scores -> ACT exp -> DVE mask for every pair of block b,
        leaving the exp'd tiles in SBUF pt tiles; PV is deferred so the PE
        never waits in-order on an exp. Returns the pt descriptors."""
        bg = list(bg)
        nj = 4 * b + 4
        pairs = [(jp, jp + 1) for jp in range(0, nj, 2)]

        def moff(j):
            t = j - 4 * b
            return t * P if t >= 1 else 0

        def sc(pi):
            j0, j1 = pairs[pi]
            ps = psscore.tile([P, 2 * SBLK], f32, tag="score")
            m0, m1 = moff(j0), moff(j1)
            q0 = slice(b * SBLK + m0, (b + 1) * SBLK)
            q1 = slice(b * SBLK + m1, (b + 1) * SBLK)
            # two PE row-groups: rows 0:64 (qk2/qk) and 64:128 (qk/qk2)
            nc.tensor.matmul(
                ps[:, m0:SBLK], qk2[:D, j0 * P : (j0 + 1) * P], qk[:D, q0],
            )
            nc.tensor.matmul(
                ps[:, SBLK + m1 :], qk[D:P, j1 * P : (j1 + 1) * P], qk2[D:P, q1],
            )
            return ps

        pts = []
        inflight = sc(0)
        for pi in range(len(pairs)):
            j0, j1 = pairs[pi]
            ps = inflight
            pt = ppool.tile([P, 2 * SBLK], MMDT, tag="pt")
            moffs = [moff(j0), moff(j1)]
            if moffs[1] < 2 * P:
                # (nearly) fully-visible pair: one batched exp over both banks
                nc.scalar.activation(pt[:], ps[:], AF.Exp, scale=float(scale))
            else:
                # strongly-masked pair: one exp over the causally-reachable
                # column range of both banks (regular 2-segment pattern)
                o = 2 * P
                psr = ps.rearrange("p (k s) -> p k s", k=2)
                ptr = pt.rearrange("p (k s) -> p k s", k=2)
                nc.scalar.activation(
                    ptr[:, :, o:], psr[:, :, o:], AF.Exp, scale=float(scale)
                )
            # only the 128-column triangle tile at the step boundary actually
            # needs masking; everything past it is fully visible
            tri = mask16[:, SBLK : SBLK + P]
            for k, j in enumerate((j0, j1)):
                t = j - 4 * b
                if t >= 0:
                    off = t * P
                    nc.vector.tensor_mul(
                        pt[:, k * SBLK + off : k * SBLK + off + P],
                        pt[:, k * SBLK + off : k * SBLK + off + P],
                        tri,
                    )
            if pi + 1 < len(pairs):
                inflight = sc(pi + 1)
            # independent next-block projection work keeps the PE busy
            # while this pair's exp runs on ScalarE
            take = max(1, (len(bg) + len(pairs) - pi - 1) // max(1, len(pairs) - pi))
            for _ in range(take):
                if bg:
                    bg.pop(0)()
            pts.append((j0, j1, moffs, pt))
        for step in bg:
            step()
        return pts

    def pv_steps(b, pts):
        """The deferred PV chain of block b plus its out(), as emit-steps to
        interleave into block b+1's scores-stream."""
        nj = 4 * b + 4
        psO = pspv.tile([P, SBLK], f32, tag="pv")

        def pv_pair(j0, j1, moffs, pt):
            for k, j in enumerate((j0, j1)):
                off = moffs[k]
                nc.tensor.matmul(
                    psO[:, off:], vAug[:, j, :],
                    pt[:, k * SBLK + off : (k + 1) * SBLK],
                    start=(j == 0), stop=(j == nj - 1),
                )

        steps = [lambda a=a: pv_pair(*a) for a in pts]
        steps.append(lambda: out(b, psO))
        return steps

    def out(b, psO):
        # rows 64:128 of psO hold the softmax denominator, pre-broadcast.
        # 1/s as exp(-ln s) on ScalarE: same ACT table set as the softmax exp.
        sl = slice(b * SBLK, (b + 1) * SBLK)
        lns = rpool.tile([D, SBLK], f32, tag="lns")
        nc.scalar.activation(lns[:], psO[D:P, :], AF.Ln)
        rcp = rpool.tile([D, SBLK], f32, tag="rcp")
        nc.scalar.activation(rcp[:], lns[:], AF.Exp, scale=-1.0)
        nc.vector.tensor_mul(yT[:, sl], psO[:D, :], rcp[:])
        if b == NSB - 1:
            # tail-critical store: halves on both HWDGE rings overlap receipts
            h0 = slice(b * SBLK, b * SBLK + SBLK // 2)
            h1 = slice(b * SBLK + SBLK // 2, (b + 1) * SBLK)
            nc.sync.dma_start(y[:, h0], yT[:, h0])
            nc.scalar.dma_start(y[:, h1], yT[:, h1])
        else:
            nc.sync.dma_start(y[:, sl], yT[:, sl])

    proj(0)
    pts_prev = None
    for b in range(NSB):
        bg = []
        if pts_prev is not None:
            bg += pv_steps(b - 1, pts_prev)
        if b + 1 < NSB:
            bg += proj_chunks(b + 1)
        pts_prev = scores_stream(b, bg=bg)
    for step in pv_steps(NSB - 1, pts_prev):
        step()


def build_nc():
    from contextlib import ExitStack

    _patch_tile_drain()
    nc = bass.Bass(target_bir_lowering=False, enable_partition_id=False)
    xt = nc.dram_tensor("xt", [P, NSB * EC * SBLK + EC * 3 * D], MMDT, kind="ExternalInput")
    y = nc.dram_tensor("y", [D, S], MMDT, kind="ExternalOutput")
    with tile.TileContext(nc) as tc:
        with ExitStack() as ctx:
            _attention(ctx, tc, xt, y)
    return nc


def make_in_maps(x, Wq, Wk, Wv):
    # wqkv_pack[p, c*3D + m] = [Wq|Wk|Wv][c*128+p, m] -> 3KB contiguous/partition
    wqkv = np.concatenate([Wq, Wk, Wv], axis=1).astype(MMNP)  # [E, 3D]
    wqkv = wqkv.reshape(EC, P, 3 * D).transpose(1, 0, 2).reshape(P, EC * 3 * D)
    x = np.asarray(x)
    maps = []
    for bc in range(B):
        # per partition: [slab0 | wqkv | slab1 | slab2 | slab3]
        xp = x[bc].astype(MMNP).reshape(NSB, SBLK, EC, P)
        xp = xp.transpose(3, 0, 2, 1).reshape(P, NSB, EC * SBLK)
        xfull = np.concatenate(
            [xp[:, 0], wqkv, xp[:, 1], xp[:, 2], xp[:, 3]], axis=1
        )
        maps.append({"xt": np.ascontiguousarray(xfull)})
    return maps


_NC = None


def kernel(x, Wq, Wk, Wv, _trace=False, _tmpdir=None):
    from concourse.bass_utils import run_bass_kernel_spmd

    global _NC
    if _NC is None:
        _NC = build_nc()
        _split_multiwaits(_NC)  # walrus-only legalization; breaks CoreSim
    in_maps = make_in_maps(x, Wq, Wk, Wv)
    res = run_bass_kernel_spmd(
        _NC, in_maps, core_ids=list(range(B)), trace=_trace, tmpdir=_tmpdir
    )
    out = np.ascontiguousarray(
        np.stack([r["y"].T for r in res.results], axis=0).astype(np.float32)
    )
    if _trace:
        kernel.last_results = res
    return out
